# revision 22
# baseline (speedup 1.0000x reference)
"""Bahdanau additive attention on 8 TRN2 NeuronCores, data-parallel over batch.

Per core (one batch element):
  w1q_T[u,t] = sum_q W1[q,u] q[t,q]      (PE, fp16 inputs, fp32 accum)
  w2k_T[u,s] = sum_v W2[v,u] v[s,v]      (PE)
  slab[u, t*128+s] = fp16(w2k_T[u,s] + w1q_T[u,t])   (DVE tensor_scalar, per-u scalar)
  slab = tanh(slab) in-place             (ACT, big-free instructions -- the bottleneck)
  scores[t,s] += sum_u scale_u tanh(...) (PE: lhsT = sliding window of zscale const)
  softmax over s (DVE+ACT, fused exp bias=-max, accum_out=rowsum)
  context = attw @ value                 (PE)
"""
import sys
import os
from contextlib import ExitStack

for _p in ("/opt/trn_rl_repo",):
    if _p not in sys.path:
        sys.path.insert(0, _p)

import numpy as np

import concourse.bass as bass
import concourse.tile as tile
from concourse import bacc, mybir
from concourse._compat import with_exitstack
from concourse.bass_utils import run_bass_kernel_spmd

F16 = mybir.dt.float16
F32 = mybir.dt.float32
AF = mybir.ActivationFunctionType
ALU = mybir.AluOpType

B = 8          # batch -> one per core
T = 128        # query positions
S = 128        # key positions
D = 1024       # feature dims (d_q = d_v = units)
NU = 8         # u tiles of 128
NQ = 8         # contraction tiles of 128
ACT_CHUNKS = 2  # tanh instructions per slab


@with_exitstack
def _build(ctx: ExitStack, tc: tile.TileContext, ins, outs):
    nc = tc.nc
    qT_d, vT_d, w1_d, w2_d, val_d, zs_d, madd_d, id_d = ins
    ctx_d, attw_d = outs

    const = ctx.enter_context(tc.tile_pool(name="const", bufs=1))
    projsb = ctx.enter_context(tc.tile_pool(name="projsb", bufs=1))
    slabs = ctx.enter_context(tc.tile_pool(name="slabs", bufs=4))
    small = ctx.enter_context(tc.tile_pool(name="small", bufs=1))
    pp = ctx.enter_context(tc.tile_pool(name="pp", bufs=2, space="PSUM"))
    sp = ctx.enter_context(tc.tile_pool(name="sp", bufs=1, space="PSUM"))
    cp = ctx.enter_context(tc.tile_pool(name="cp", bufs=1, space="PSUM"))

    # ---- const DMAs: w1 path and w2 path on separate queues so proj(0)
    # (which needs ALL of w1) isn't serialized behind w2 chunks ----
    qT = const.tile([128, D], F16)
    vT = const.tile([128, D], F16)
    w1 = const.tile([128, NQ * D], F16)
    w2 = const.tile([128, NQ * D], F16)
    # first k-tile of qT and w1 as tiny DMAs so proj(0) starts ASAP
    nc.sync.dma_start(qT[:, 0:128], qT_d[:, 0:128])
    nc.sync.dma_start(w1[:, 0:128], w1_d[:, 0:128])
    nc.scalar.dma_start(vT[:, 0:128], vT_d[:, 0:128])
    nc.scalar.dma_start(w2[:, 0:128], w2_d[:, 0:128])
    nc.sync.dma_start(qT[:, 128:D], qT_d[:, 128:D])
    nc.sync.dma_start(w1[:, 128:D], w1_d[:, 128:D])
    nc.scalar.dma_start(vT[:, 128:D], vT_d[:, 128:D])
    nc.scalar.dma_start(w2[:, 128:D], w2_d[:, 128:D])
    for ut in range(1, NU):
        nc.sync.dma_start(w1[:, ut * D:(ut + 1) * D], w1_d[:, ut * D:(ut + 1) * D])
        nc.scalar.dma_start(w2[:, ut * D:(ut + 1) * D], w2_d[:, ut * D:(ut + 1) * D])
    val = const.tile([128, D], F16)
    nc.gpsimd.dma_start(val[:], val_d[:])
    zs = const.tile([128, NU * 256], F16)
    nc.gpsimd.dma_start(zs[:], zs_d[:])
    madd = const.tile([128, S], F32)
    nc.gpsimd.dma_start(madd[:], madd_d[:])
    ident = const.tile([128, 128], F16)
    nc.gpsimd.dma_start(ident[:], id_d[:])

    DUP = 4
    w1q4 = projsb.tile([128, D * DUP], F16)  # [u_local, (ut*128 + t)*4 + k]: w1q dup x4
    w2k = projsb.tile([128, D], F16)         # [u_local, ut*128 + s]

    def proj(ut):
        psA = pp.tile([128, 128], F32, tag="proj")
        for qt in range(NQ):
            nc.tensor.matmul(
                psA[:], w1[:, ut * D + qt * 128: ut * D + (qt + 1) * 128],
                qT[:, qt * 128:(qt + 1) * 128],
                start=(qt == 0), stop=(qt == NQ - 1))
        o = w1q4[:, ut * 128 * DUP:(ut + 1) * 128 * DUP]
        nc.vector.tensor_copy(o.rearrange("p (t k) -> p t k", k=DUP),
                              psA[:].unsqueeze(2).to_broadcast((128, 128, DUP)))
        psB = pp.tile([128, 128], F32, tag="proj")
        for qt in range(NQ):
            nc.tensor.matmul(
                psB[:], w2[:, ut * D + qt * 128: ut * D + (qt + 1) * 128],
                vT[:, qt * 128:(qt + 1) * 128],
                start=(qt == 0), stop=(qt == NQ - 1))
        nc.vector.tensor_copy(w2k[:, ut * 128:(ut + 1) * 128], psB[:])

    proj(0)

    scores_psA = sp.tile([128, S], F32)   # rows 0..63 live here (bank A)
    scores_psB = sp.tile([128, S], F32)   # rows 64..127 live here (bank B)

    tail_half = _make_tail(nc, small, cp, sp, madd, ident, val, attw_d, ctx_d,
                           scores_psA, scores_psB)

    def emit_reduce(ut, ta, tb, slab):
        for t in range(ta, tb):
            half = t // 64
            tile_ = scores_psA if half == 0 else scores_psB
            base = half * 64
            off = ut * 256 + 127 - (t % 64)
            nc.tensor.matmul(tile_[base:base + 64, :], zs[:, off:off + 64],
                             slab[:, t * S:(t + 1) * S],
                             start=(ut == 0 and t % 64 == 0),
                             stop=(ut == NU - 1 and t % 64 == 63))

    TB = 32  # t's per pairwise-add instruction
    for ut in range(NU):
        slab = slabs.tile([128, T * S], F16)
        # pairwise sums, 2x_1P packed mode: innermost (step1, DUP) on all APs
        tranges = [(0, 8), (8, 32), (32, 64), (64, 96), (96, 128)] if ut == 0 \
            else [(t0, t0 + TB) for t0 in range(0, T, TB)]
        for t0, t1 in tranges:
            tb = t1 - t0
            in0 = w2k[:, ut * 128:(ut + 1) * 128] \
                .rearrange("p (s k) -> p s k", k=DUP) \
                .unsqueeze(1).to_broadcast((128, tb, S // DUP, DUP))
            in1 = w1q4[:, (ut * 128 + t0) * DUP:(ut * 128 + t1) * DUP] \
                .rearrange("p (t k) -> p t k", k=DUP) \
                .unsqueeze(2).to_broadcast((128, tb, S // DUP, DUP))
            out3 = slab[:, t0 * S:t1 * S] \
                .rearrange("p (t s k) -> p t s k", t=tb, k=DUP)
            nc.vector.tensor_tensor(out3, in0, in1, ALU.add)
        # tanh in place (finer chunks on the last slab so its reduce
        # matmuls start sooner -- they are the serial tail)
        if ut == 0:
            bounds = [0, 8, 32, 64, 128]
        elif ut == NU - 1:
            bounds = [0, 64, 96, 112, 128]
        else:
            bounds = [0, 128] if ut > 1 else [0, 64, 128]
        for c0, c1 in zip(bounds[:-1], bounds[1:]):
            nc.scalar.activation(slab[:, c0 * S:c1 * S],
                                 slab[:, c0 * S:c1 * S], AF.Tanh)
        # keep PE warm: next projection before this slab's reduce
        if ut + 1 < NU:
            proj(ut + 1)
        # scores[t, s] += sum_u scale_u * slab[u, t*128+s]
        if ut < NU - 1:
            emit_reduce(ut, 0, T, slab)
        else:
            emit_reduce(ut, 0, 64, slab)
            tail_half(0)
            emit_reduce(ut, 64, T, slab)
            tail_half(1)

    # ---- outputs already emitted via tail_half ----

def _make_tail(nc, small, cp, sp, madd, ident, val, attw_d, ctx_d,
               scores_psA, scores_psB):
    sm = small.tile([128, S], F32)
    rmax = small.tile([128, 1], F32)
    nmax = small.tile([128, 1], F32)
    probs = small.tile([128, S], F32)
    rsum = small.tile([128, 1], F32)
    rinv = small.tile([128, 1], F32)
    attw = small.tile([128, S], F32)
    attwh = small.tile([128, S], F16)
    attwT = small.tile([128, T], F16)
    ctx_ps = cp.tile([128, D], F32)
    ctx_sb = small.tile([128, D], F16)

    def tail_half(h):
        p0, p1 = h * 64, h * 64 + 64
        sc = (scores_psA if h == 0 else scores_psB)[p0:p1, :]
        nc.vector.tensor_tensor(sm[p0:p1, :], sc, madd[p0:p1, :], ALU.add)
        nc.vector.reduce_max(rmax[p0:p1, :], sm[p0:p1, :], axis=mybir.AxisListType.X)
        nc.vector.tensor_scalar(nmax[p0:p1, :], rmax[p0:p1, :], -1.0, None, ALU.mult)
        nc.scalar.activation(probs[p0:p1, :], sm[p0:p1, :], AF.Exp,
                             bias=nmax[p0:p1, :], accum_out=rsum[p0:p1, :])
        nc.vector.reciprocal(rinv[p0:p1, :], rsum[p0:p1, :])
        nc.vector.tensor_scalar(attwh[p0:p1, :], probs[p0:p1, :], rinv[p0:p1, :],
                                None, ALU.mult)
        nc.vector.tensor_scalar(attw[p0:p1, :], probs[p0:p1, :], rinv[p0:p1, :],
                                None, ALU.mult)
        nc.sync.dma_start(attw_d[p0:p1, :], attw[p0:p1, :])
        trp = sp.tile([128, 64], F16, tag="trp")
        nc.tensor.transpose(trp[:], attwh[p0:p1, :], ident[p0:p1, p0:p1])
        nc.vector.tensor_copy(attwT[:, p0:p1], trp[:])
        for n in range(2):
            nc.tensor.matmul(ctx_ps[p0:p1, n * 512:(n + 1) * 512],
                             attwT[:, p0:p1],
                             val[:, n * 512:(n + 1) * 512],
                             start=True, stop=True)
        nc.vector.tensor_copy(ctx_sb[p0:p1, 0:512], ctx_ps[p0:p1, 0:512])
        nc.sync.dma_start(ctx_d[p0:p1, 0:512], ctx_sb[p0:p1, 0:512])
        nc.scalar.copy(ctx_sb[p0:p1, 512:1024], ctx_ps[p0:p1, 512:1024])
        nc.scalar.dma_start(ctx_d[p0:p1, 512:1024], ctx_sb[p0:p1, 512:1024])

    return tail_half


_CACHE = {}


def _get_nc():
    if "nc" in _CACHE:
        return _CACHE["nc"]
    nc = bacc.Bacc("TRN2", target_bir_lowering=False, debug=False,
                   num_devices=B)
    ins = [
        nc.dram_tensor("qT", (128, D), F16, kind="ExternalInput").ap(),
        nc.dram_tensor("vT", (128, D), F16, kind="ExternalInput").ap(),
        nc.dram_tensor("w1", (128, NQ * D), F16, kind="ExternalInput").ap(),
        nc.dram_tensor("w2", (128, NQ * D), F16, kind="ExternalInput").ap(),
        nc.dram_tensor("val", (128, D), F16, kind="ExternalInput").ap(),
        nc.dram_tensor("zs", (128, NU * 256), F16, kind="ExternalInput").ap(),
        nc.dram_tensor("madd", (128, S), F32, kind="ExternalInput").ap(),
        nc.dram_tensor("ident", (128, 128), F16, kind="ExternalInput").ap(),
    ]
    outs = [
        nc.dram_tensor("ctx_out", (T, D), F16, kind="ExternalOutput").ap(),
        nc.dram_tensor("attw_out", (T, S), F32, kind="ExternalOutput").ap(),
    ]
    with tile.TileContext(nc) as tc:
        _build(tc, ins, outs)
    nc.compile()
    _CACHE["nc"] = nc
    return nc


def make_in_maps(query, value, mask, W1, W2, scale):
    w1_h = np.ascontiguousarray(
        W1.reshape(NQ, 128, NU, 128).transpose(1, 2, 0, 3).reshape(128, NQ * D)
    ).astype(np.float16)
    w2_h = np.ascontiguousarray(
        W2.reshape(NQ, 128, NU, 128).transpose(1, 2, 0, 3).reshape(128, NQ * D)
    ).astype(np.float16)
    zs_h = np.zeros((128, NU * 256), np.float16)
    for ut in range(NU):
        zs_h[:, ut * 256 + 127] = scale[ut * 128:(ut + 1) * 128].astype(np.float16)
    id_h = np.eye(128, dtype=np.float16)
    in_maps = []
    for b in range(B):
        qT_h = np.ascontiguousarray(
            query[b].T.reshape(NQ, 128, T).transpose(1, 0, 2).reshape(128, NQ * T)
        ).astype(np.float16)
        vT_h = np.ascontiguousarray(
            value[b].T.reshape(NQ, 128, S).transpose(1, 0, 2).reshape(128, NQ * S)
        ).astype(np.float16)
        val_h = value[b].astype(np.float16)
        madd_h = np.ascontiguousarray(np.broadcast_to(
            np.where(mask[b], np.float32(0.0), np.float32(-1e9))[None, :],
            (T, S))).astype(np.float32)
        in_maps.append({
            "qT": qT_h, "vT": vT_h, "w1": w1_h, "w2": w2_h, "val": val_h,
            "zs": zs_h, "madd": madd_h, "ident": id_h,
        })
    return in_maps


def kernel(query, value, mask, W1, W2, scale):
    query = np.asarray(query, np.float32)
    value = np.asarray(value, np.float32)
    mask = np.asarray(mask, bool)
    W1 = np.asarray(W1, np.float32)
    W2 = np.asarray(W2, np.float32)
    scale = np.asarray(scale, np.float32)

    nc = _get_nc()
    in_maps = make_in_maps(query, value, mask, W1, W2, scale)
    res = run_bass_kernel_spmd(nc, in_maps, list(range(B)),
                               trace=os.environ.get("BASS_TRACE", "") == "1")
    _CACHE["last_result"] = res
    ctx = np.stack([res.results[b]["ctx_out"] for b in range(B)]).astype(np.float32)
    attw = np.stack([res.results[b]["attw_out"] for b in range(B)]).astype(np.float32)
    return ctx, attw


# revision 23
# speedup vs baseline: 1.0465x; 1.0465x over previous
"""Bahdanau additive attention on 8 TRN2 NeuronCores, data-parallel over batch.

Per core (one batch element):
  w1q_T[u,t] = sum_q W1[q,u] q[t,q]      (PE, fp16 inputs, fp32 accum)
  w2k_T[u,s] = sum_v W2[v,u] v[s,v]      (PE)
  slab[u, t*128+s] = fp16(w2k_T[u,s] + w1q_T[u,t])   (DVE tensor_scalar, per-u scalar)
  slab = tanh(slab) in-place             (ACT, big-free instructions -- the bottleneck)
  scores[t,s] += sum_u scale_u tanh(...) (PE: lhsT = sliding window of zscale const)
  softmax over s (DVE+ACT, fused exp bias=-max, accum_out=rowsum)
  context = attw @ value                 (PE)
"""
import sys
import os
from contextlib import ExitStack

for _p in ("/opt/trn_rl_repo",):
    if _p not in sys.path:
        sys.path.insert(0, _p)

import numpy as np

import concourse.bass as bass
import concourse.tile as tile
from concourse import bacc, mybir
from concourse._compat import with_exitstack
from concourse.bass_utils import run_bass_kernel_spmd

F16 = mybir.dt.float16
F32 = mybir.dt.float32
AF = mybir.ActivationFunctionType
ALU = mybir.AluOpType

B = 8          # batch -> one per core
T = 128        # query positions
S = 128        # key positions
D = 1024       # feature dims (d_q = d_v = units)
NU = 8         # u tiles of 128
NQ = 8         # contraction tiles of 128
ACT_CHUNKS = 2  # tanh instructions per slab


@with_exitstack
def _build(ctx: ExitStack, tc: tile.TileContext, ins, outs):
    nc = tc.nc
    qT_d, vT_d, w1_d, w2_d, val_d, zs_d, madd_d, id_d = ins
    ctx_d, attw_d = outs

    const = ctx.enter_context(tc.tile_pool(name="const", bufs=1))
    projsb = ctx.enter_context(tc.tile_pool(name="projsb", bufs=1))
    slabs = ctx.enter_context(tc.tile_pool(name="slabs", bufs=4))
    small = ctx.enter_context(tc.tile_pool(name="small", bufs=1))
    pp = ctx.enter_context(tc.tile_pool(name="pp", bufs=2, space="PSUM"))
    sp = ctx.enter_context(tc.tile_pool(name="sp", bufs=1, space="PSUM"))
    cp = ctx.enter_context(tc.tile_pool(name="cp", bufs=1, space="PSUM"))

    # ---- const DMAs: w1 path and w2 path on separate queues so proj(0)
    # (which needs ALL of w1) isn't serialized behind w2 chunks ----
    qT = const.tile([128, D], F16)
    vT = const.tile([128, D], F16)
    w1 = const.tile([128, NQ * D], F16)
    w2 = const.tile([128, NQ * D], F16)
    # first k-tile of qT and w1 as tiny DMAs so proj(0) starts ASAP
    nc.sync.dma_start(qT[:, 0:128], qT_d[:, 0:128])
    nc.sync.dma_start(w1[:, 0:128], w1_d[:, 0:128])
    nc.gpsimd.dma_start(vT[:, 0:128], vT_d[:, 0:128])
    nc.gpsimd.dma_start(w2[:, 0:128], w2_d[:, 0:128])
    nc.sync.dma_start(qT[:, 128:D], qT_d[:, 128:D])
    nc.sync.dma_start(w1[:, 128:D], w1_d[:, 128:D])
    nc.gpsimd.dma_start(vT[:, 128:D], vT_d[:, 128:D])
    nc.gpsimd.dma_start(w2[:, 128:D], w2_d[:, 128:D])
    for ut in range(1, NU):
        nc.sync.dma_start(w1[:, ut * D:(ut + 1) * D], w1_d[:, ut * D:(ut + 1) * D])
        nc.gpsimd.dma_start(w2[:, ut * D:(ut + 1) * D], w2_d[:, ut * D:(ut + 1) * D])
    # tail-only consts ride the sync queue AFTER the critical w1 chunks
    val = const.tile([128, D], F16)
    zs = const.tile([128, NU * 256], F16)
    madd = const.tile([128, S], F32)
    ident = const.tile([128, 128], F16)
    nc.sync.dma_start(zs[:], zs_d[:])
    nc.sync.dma_start(val[:], val_d[:])
    nc.sync.dma_start(madd[:], madd_d[:])
    nc.sync.dma_start(ident[:], id_d[:])

    DUP = 4
    w1q4 = projsb.tile([128, D * DUP], F16)  # [u_local, (ut*128 + t)*4 + k]: w1q dup x4
    w2k = projsb.tile([128, D], F16)         # [u_local, ut*128 + s]

    def proj(ut):
        psA = pp.tile([128, 128], F32, tag="proj")
        for qt in range(NQ):
            nc.tensor.matmul(
                psA[:], w1[:, ut * D + qt * 128: ut * D + (qt + 1) * 128],
                qT[:, qt * 128:(qt + 1) * 128],
                start=(qt == 0), stop=(qt == NQ - 1))
        o = w1q4[:, ut * 128 * DUP:(ut + 1) * 128 * DUP]
        nc.vector.tensor_copy(o.rearrange("p (t k) -> p t k", k=DUP),
                              psA[:].unsqueeze(2).to_broadcast((128, 128, DUP)))
        psB = pp.tile([128, 128], F32, tag="proj")
        for qt in range(NQ):
            nc.tensor.matmul(
                psB[:], w2[:, ut * D + qt * 128: ut * D + (qt + 1) * 128],
                vT[:, qt * 128:(qt + 1) * 128],
                start=(qt == 0), stop=(qt == NQ - 1))
        nc.vector.tensor_copy(w2k[:, ut * 128:(ut + 1) * 128], psB[:])

    proj(0)

    scores_psA = sp.tile([128, S], F32)   # rows 0..63 live here (bank A)
    scores_psB = sp.tile([128, S], F32)   # rows 64..127 live here (bank B)

    tail_half = _make_tail(nc, small, cp, sp, madd, ident, val, attw_d, ctx_d,
                           scores_psA, scores_psB)

    def emit_reduce(ut, ta, tb, slab):
        for t in range(ta, tb):
            half = t // 64
            tile_ = scores_psA if half == 0 else scores_psB
            base = half * 64
            off = ut * 256 + 127 - (t % 64)
            nc.tensor.matmul(tile_[base:base + 64, :], zs[:, off:off + 64],
                             slab[:, t * S:(t + 1) * S],
                             start=(ut == 0 and t % 64 == 0),
                             stop=(ut == NU - 1 and t % 64 == 63))

    TB = 32  # t's per pairwise-add instruction
    for ut in range(NU):
        slab = slabs.tile([128, T * S], F16)
        # pairwise sums, 2x_1P packed mode: innermost (step1, DUP) on all APs
        tranges = [(0, 8), (8, 32), (32, 64), (64, 96), (96, 128)] if ut == 0 \
            else [(t0, t0 + TB) for t0 in range(0, T, TB)]
        for t0, t1 in tranges:
            tb = t1 - t0
            in0 = w2k[:, ut * 128:(ut + 1) * 128] \
                .rearrange("p (s k) -> p s k", k=DUP) \
                .unsqueeze(1).to_broadcast((128, tb, S // DUP, DUP))
            in1 = w1q4[:, (ut * 128 + t0) * DUP:(ut * 128 + t1) * DUP] \
                .rearrange("p (t k) -> p t k", k=DUP) \
                .unsqueeze(2).to_broadcast((128, tb, S // DUP, DUP))
            out3 = slab[:, t0 * S:t1 * S] \
                .rearrange("p (t s k) -> p t s k", t=tb, k=DUP)
            nc.vector.tensor_tensor(out3, in0, in1, ALU.add)
        # tanh in place (finer chunks on the last slab so its reduce
        # matmuls start sooner -- they are the serial tail)
        if ut == 0:
            bounds = [0, 8, 32, 64, 128]
        elif ut == NU - 1:
            bounds = [0, 64, 96, 112, 128]
        else:
            bounds = [0, 128] if ut > 1 else [0, 64, 128]
        for c0, c1 in zip(bounds[:-1], bounds[1:]):
            nc.scalar.activation(slab[:, c0 * S:c1 * S],
                                 slab[:, c0 * S:c1 * S], AF.Tanh)
        # keep PE warm: next projection before this slab's reduce
        if ut + 1 < NU:
            proj(ut + 1)
        # scores[t, s] += sum_u scale_u * slab[u, t*128+s]
        if ut < NU - 1:
            emit_reduce(ut, 0, T, slab)
        else:
            emit_reduce(ut, 0, 64, slab)
            tail_half(0)
            emit_reduce(ut, 64, T, slab)
            tail_half(1)

    # ---- outputs already emitted via tail_half ----

def _make_tail(nc, small, cp, sp, madd, ident, val, attw_d, ctx_d,
               scores_psA, scores_psB):
    sm = small.tile([128, S], F32)
    rmax = small.tile([128, 1], F32)
    nmax = small.tile([128, 1], F32)
    probs = small.tile([128, S], F32)
    rsum = small.tile([128, 1], F32)
    rinv = small.tile([128, 1], F32)
    attw = small.tile([128, S], F32)
    attwh = small.tile([128, S], F16)
    attwT = small.tile([128, T], F16)
    ctx_ps = cp.tile([128, D], F32)
    ctx_sb = small.tile([128, D], F16)

    def tail_half(h):
        p0, p1 = h * 64, h * 64 + 64
        sc = (scores_psA if h == 0 else scores_psB)[p0:p1, :]
        nc.vector.tensor_tensor(sm[p0:p1, :], sc, madd[p0:p1, :], ALU.add)
        nc.vector.reduce_max(rmax[p0:p1, :], sm[p0:p1, :], axis=mybir.AxisListType.X)
        nc.vector.tensor_scalar(nmax[p0:p1, :], rmax[p0:p1, :], -1.0, None, ALU.mult)
        nc.scalar.activation(probs[p0:p1, :], sm[p0:p1, :], AF.Exp,
                             bias=nmax[p0:p1, :], accum_out=rsum[p0:p1, :])
        nc.vector.reciprocal(rinv[p0:p1, :], rsum[p0:p1, :])
        nc.vector.tensor_scalar(attwh[p0:p1, :], probs[p0:p1, :], rinv[p0:p1, :],
                                None, ALU.mult)
        nc.vector.tensor_scalar(attw[p0:p1, :], probs[p0:p1, :], rinv[p0:p1, :],
                                None, ALU.mult)
        nc.sync.dma_start(attw_d[p0:p1, :], attw[p0:p1, :])
        trp = sp.tile([128, 64], F16, tag="trp")
        nc.tensor.transpose(trp[:], attwh[p0:p1, :], ident[p0:p1, p0:p1])
        nc.vector.tensor_copy(attwT[:, p0:p1], trp[:])
        for n in range(2):
            nc.tensor.matmul(ctx_ps[p0:p1, n * 512:(n + 1) * 512],
                             attwT[:, p0:p1],
                             val[:, n * 512:(n + 1) * 512],
                             start=True, stop=True)
        nc.vector.tensor_copy(ctx_sb[p0:p1, 0:512], ctx_ps[p0:p1, 0:512])
        nc.sync.dma_start(ctx_d[p0:p1, 0:512], ctx_sb[p0:p1, 0:512])
        nc.scalar.copy(ctx_sb[p0:p1, 512:1024], ctx_ps[p0:p1, 512:1024])
        nc.scalar.dma_start(ctx_d[p0:p1, 512:1024], ctx_sb[p0:p1, 512:1024])

    return tail_half


_CACHE = {}


def _get_nc():
    if "nc" in _CACHE:
        return _CACHE["nc"]
    nc = bacc.Bacc("TRN2", target_bir_lowering=False, debug=False,
                   num_devices=B)
    ins = [
        nc.dram_tensor("qT", (128, D), F16, kind="ExternalInput").ap(),
        nc.dram_tensor("vT", (128, D), F16, kind="ExternalInput").ap(),
        nc.dram_tensor("w1", (128, NQ * D), F16, kind="ExternalInput").ap(),
        nc.dram_tensor("w2", (128, NQ * D), F16, kind="ExternalInput").ap(),
        nc.dram_tensor("val", (128, D), F16, kind="ExternalInput").ap(),
        nc.dram_tensor("zs", (128, NU * 256), F16, kind="ExternalInput").ap(),
        nc.dram_tensor("madd", (128, S), F32, kind="ExternalInput").ap(),
        nc.dram_tensor("ident", (128, 128), F16, kind="ExternalInput").ap(),
    ]
    outs = [
        nc.dram_tensor("ctx_out", (T, D), F16, kind="ExternalOutput").ap(),
        nc.dram_tensor("attw_out", (T, S), F32, kind="ExternalOutput").ap(),
    ]
    with tile.TileContext(nc) as tc:
        _build(tc, ins, outs)
    nc.compile()
    _CACHE["nc"] = nc
    return nc


def make_in_maps(query, value, mask, W1, W2, scale):
    w1_h = np.ascontiguousarray(
        W1.reshape(NQ, 128, NU, 128).transpose(1, 2, 0, 3).reshape(128, NQ * D)
    ).astype(np.float16)
    w2_h = np.ascontiguousarray(
        W2.reshape(NQ, 128, NU, 128).transpose(1, 2, 0, 3).reshape(128, NQ * D)
    ).astype(np.float16)
    zs_h = np.zeros((128, NU * 256), np.float16)
    for ut in range(NU):
        zs_h[:, ut * 256 + 127] = scale[ut * 128:(ut + 1) * 128].astype(np.float16)
    id_h = np.eye(128, dtype=np.float16)
    in_maps = []
    for b in range(B):
        qT_h = np.ascontiguousarray(
            query[b].T.reshape(NQ, 128, T).transpose(1, 0, 2).reshape(128, NQ * T)
        ).astype(np.float16)
        vT_h = np.ascontiguousarray(
            value[b].T.reshape(NQ, 128, S).transpose(1, 0, 2).reshape(128, NQ * S)
        ).astype(np.float16)
        val_h = value[b].astype(np.float16)
        madd_h = np.ascontiguousarray(np.broadcast_to(
            np.where(mask[b], np.float32(0.0), np.float32(-1e9))[None, :],
            (T, S))).astype(np.float32)
        in_maps.append({
            "qT": qT_h, "vT": vT_h, "w1": w1_h, "w2": w2_h, "val": val_h,
            "zs": zs_h, "madd": madd_h, "ident": id_h,
        })
    return in_maps


def kernel(query, value, mask, W1, W2, scale):
    query = np.asarray(query, np.float32)
    value = np.asarray(value, np.float32)
    mask = np.asarray(mask, bool)
    W1 = np.asarray(W1, np.float32)
    W2 = np.asarray(W2, np.float32)
    scale = np.asarray(scale, np.float32)

    nc = _get_nc()
    in_maps = make_in_maps(query, value, mask, W1, W2, scale)
    res = run_bass_kernel_spmd(nc, in_maps, list(range(B)),
                               trace=os.environ.get("BASS_TRACE", "") == "1")
    _CACHE["last_result"] = res
    ctx = np.stack([res.results[b]["ctx_out"] for b in range(B)]).astype(np.float32)
    attw = np.stack([res.results[b]["attw_out"] for b in range(B)]).astype(np.float32)
    return ctx, attw
